# revision 35
# baseline (speedup 1.0000x reference)
"""2-layer GCN (segment-sum message passing) on 8 trn2 NeuronCores.

Math (from the reference):
    row/col have self-loops appended with weight 5 (= trunc(log2(E/N))).
    deg[i] = (# real edges with row==i) + 5 ;  dis = 1/sqrt(deg)
    norm[e] = dis[row]*w*dis[col]  (w=1 for real edges)
    gcn(h, W): agg[d] = sum_{e: row=d} norm[e] * (h@W)[col[e]]
    out = (relu(gcn(x,W1)) -> gcn(.,W2)) @ W3.T + b3

Factorization used here: with zs = dis * (h@W),
    gcn(h,W)[d] = dis[d] * ( sum_{e: row=d} zs[col[e]]  +  5*zs[d] )
so the per-edge work is a pure gather of zs rows plus a segment-sum, with no
per-edge scaling. The segment-sum runs on the tensor engine as
Sel.T @ msg where Sel[e,:] is the one-hot of the edge's destination-within-
tile, built on-chip by tensor_scalar(is_equal) against an iota constant.
The 5*zs[d] self-loop term is a diag(5) matmul into the same PSUM
accumulator.

Sharding: destination nodes split across 8 cores; zs tables are
all-gathered between layers; gathers read the fp16 table via dma_gather
(int16 indices, 4 source-range buckets to stay under the 32768-row limit).
"""

import math
import os
import sys

sys.path.insert(0, "/opt/trn_rl_repo")

import numpy as np

import concourse.bass as bass  # noqa: F401
import concourse.tile as tile
from concourse import bacc, mybir
from concourse.bass_utils import run_bass_kernel_spmd

NCORES = 8
SG_TILES = 6  # dst tiles per supergroup (PSUM agg banks)
SEL_PATTERN = os.environ.get("GCN_SEL_PATTERN", "vvs")  # v=DVE s=ACT(any) p=POOL
F16 = mybir.dt.float16
F32 = mybir.dt.float32


def _ceil(a, b):
    return -(-a // b)


def preprocess(x, edge_index, W1, W2, W3, b3):
    """Host-side layout/index prep. Returns (meta, in_maps)."""
    N, F_IN = x.shape
    HID = W1.shape[1]
    NCLS = W3.shape[0]
    E = edge_index.shape[1]

    npc_raw = _ceil(N, NCORES)  # nodes per core (unpadded)
    NPC = _ceil(npc_raw, 128) * 128  # padded per-core rows
    NTILES = NPC // 128
    NTAB = NCORES * NPC  # table rows
    NBUCKETS = _ceil(NTAB, 25088)
    BUCKET_ROWS = _ceil(_ceil(NTAB, NBUCKETS), 128) * 128
    assert BUCKET_ROWS <= 32768
    NSG = _ceil(NTILES, SG_TILES)
    KIN = F_IN // 128  # contraction chunks for layer-1 matmul

    row = np.asarray(edge_index[0], np.int64)
    col = np.asarray(edge_index[1], np.int64)

    fill = float(math.trunc(math.log2(E / N)))
    deg = np.bincount(row, minlength=N).astype(np.float64) + fill
    dis = (1.0 / np.sqrt(deg)).astype(np.float32)
    dis_pad = np.zeros(NTAB, np.float32)
    tab_row_of_node = (np.arange(N) // npc_raw) * NPC + (np.arange(N) % npc_raw)
    dis_pad[tab_row_of_node] = dis

    # per-edge placement
    e_core = row // npc_raw
    e_local = row % npc_raw
    e_tile = e_local // 128
    e_dstloc = (e_local % 128).astype(np.float32)
    src_tab = (col // npc_raw) * NPC + (col % npc_raw)
    e_bucket = src_tab // BUCKET_ROWS
    e_idx16 = (src_tab % BUCKET_ROWS).astype(np.int16)

    # stream order per core: supergroup -> bucket -> tile-within-sg -> edges
    e_sg = e_tile // SG_TILES
    e_tsg = e_tile % SG_TILES
    group = ((e_sg * NBUCKETS + e_bucket) * SG_TILES + e_tsg).astype(np.int64)
    NGRP = NSG * NBUCKETS * SG_TILES

    # counts[c, g]
    counts = np.zeros((NCORES, NGRP), np.int64)
    flat = e_core * NGRP + group
    bc = np.bincount(flat, minlength=NCORES * NGRP)
    counts = bc.reshape(NCORES, NGRP)
    gcnt = counts.max(axis=0)  # uniform padded counts (in edges)
    gchunks = _ceil(gcnt, 128)  # chunks per group (uniform)
    gpad = gchunks * 128

    # group -> (sg, b, tsg); chunk offsets
    goff = np.zeros(NGRP + 1, np.int64)
    np.cumsum(gpad, out=goff[1:])
    TOT_IDX = int(goff[-1])
    TOT_CHUNKS = TOT_IDX // 128

    # assemble per-core streams
    order = np.argsort(flat, kind="stable")
    run_starts = np.concatenate(([0], np.cumsum(bc)[:-1]))
    within = np.empty(E, np.int64)
    within[order] = np.arange(E) - run_starts[flat[order]]
    # position of edge e in its group's output block
    pos = goff[group] + within
    idx_stream = np.zeros((NCORES, TOT_IDX), np.int16)
    dloc_stream = np.full((NCORES, TOT_IDX), -64.0, np.float32)
    idx_stream[e_core, pos] = e_idx16
    dloc_stream[e_core, pos] = e_dstloc

    # wrapped idx layout: within each gather call (= (sg,b) run of groups),
    # idx j -> [j%16, call_off/16 + j//16]. Calls are contiguous (sg,b)
    # blocks of SG_TILES groups, so wrapping per 16 within the whole stream
    # works as long as each call's length %16 == 0 (it is: %128).
    # call boundaries:
    # walrus caps one gather at 8192 indices (64 chunks): split bigger calls.
    MAXCH = 16
    calls = []  # (sg, b, idx_off, chunk_off, [(tile, nchunks), ...])
    for s in range(NSG):
        tiles = list(range(s * SG_TILES, min(NTILES, (s + 1) * SG_TILES)))
        for b in range(NBUCKETS):
            g0 = (s * NBUCKETS + b) * SG_TILES
            idx_off = int(goff[g0])
            assert idx_off % 128 == 0
            pieces = []  # flat (tile, nchunks) honoring MAXCH
            cur = []
            cur_n = 0
            for i, t in enumerate(tiles):
                n = int(gchunks[g0 + i])
                while n > 0:
                    take = min(n, MAXCH - cur_n)
                    if take > 0:
                        cur.append((t, take))
                        cur_n += take
                        n -= take
                    if cur_n == MAXCH:
                        pieces.append(cur)
                        cur = []
                        cur_n = 0
            if cur:
                pieces.append(cur)
            off = idx_off
            for tl in pieces:
                calls.append((s, b, off, off // 128, tl))
                off += sum(nn for _, nn in tl) * 128
            assert off == int(goff[g0 + len(tiles)])

    idx_wrapped = np.zeros((NCORES, 128, TOT_IDX // 16), np.int16)
    for s, b, io, co, tl in calls:
        L = sum(n for _, n in tl) * 128
        if L == 0:
            continue
        blk = idx_stream[:, io : io + L].reshape(NCORES, L // 16, 16)
        blk = np.swapaxes(blk, 1, 2)  # [NCORES, 16, L/16]
        idx_wrapped[:, :, io // 16 : (io + L) // 16] = np.tile(blk, (1, 8, 1))

    dlocT = np.ascontiguousarray(
        np.swapaxes(dloc_stream.reshape(NCORES, TOT_CHUNKS, 128), 1, 2)
    )  # [NCORES, 128, TOT_CHUNKS]

    # host-precomputed Sel one-hot stream: per call, layout [128 edge-lanes,
    # nch*128 dst-cols] contiguous so each call DMAs one contiguous blob.
    eye = np.eye(128, dtype=np.float16)
    zero_row = np.zeros(128, np.float16)
    sel_rows = np.concatenate([eye, zero_row[None, :]])  # [-1] -> zeros
    dl_int = np.where(
        dloc_stream < 0, 128, dloc_stream.astype(np.int64)
    )  # [NCORES, TOT_IDX]
    sel_full = sel_rows[dl_int]  # [NCORES, TOT_IDX, 128] fp16
    sel_full = sel_full.reshape(NCORES, TOT_CHUNKS, 128, 128)
    selT = np.ascontiguousarray(
        np.swapaxes(sel_full, 1, 2).reshape(NCORES, 128, TOT_CHUNKS * 128)
    )
    del sel_full

    # per-tile chunk totals (for start/stop flags)
    tile_chunks = np.zeros(NTILES, np.int64)
    for s, b, io, co, tl in calls:
        for t, n in tl:
            tile_chunks[t] += n

    # dense inputs
    xpad = np.zeros((NTAB, F_IN), np.float32)
    xpad[tab_row_of_node] = np.asarray(x, np.float32)
    xT = np.ascontiguousarray(
        np.swapaxes(xpad.reshape(NCORES, NPC, F_IN), 1, 2).astype(np.float16)
    )  # [NCORES, F_IN, NPC]

    disT = np.ascontiguousarray(
        np.swapaxes(dis_pad.reshape(NCORES, NTILES, 128), 1, 2)
    )  # [NCORES, 128, NTILES]

    iota_np = np.tile(np.arange(128, dtype=np.float16)[None, :], (128, 1))
    diag5_np = (fill * np.eye(128)).astype(np.float16)
    id_np = np.eye(128, dtype=np.float16)
    W1_np = np.ascontiguousarray(
        np.asarray(W1, np.float32).reshape(KIN, 128, HID).astype(np.float16)
    )
    W2_np = np.asarray(W2, np.float32).astype(np.float16)
    W3T_np = np.ascontiguousarray(np.asarray(W3, np.float32).T.astype(np.float16))
    b3_np = np.asarray(b3, np.float32).reshape(NCLS, 1)

    meta = dict(
        N=N,
        F_IN=F_IN,
        HID=HID,
        NCLS=NCLS,
        NPC=NPC,
        npc_raw=npc_raw,
        NTILES=NTILES,
        NTAB=NTAB,
        NBUCKETS=NBUCKETS,
        BUCKET_ROWS=BUCKET_ROWS,
        NSG=NSG,
        KIN=KIN,
        TOT_IDX=TOT_IDX,
        TOT_CHUNKS=TOT_CHUNKS,
        MAXCH=MAXCH,
        calls=calls,
        tile_chunks=tile_chunks,
    )
    in_maps = []
    for c in range(NCORES):
        in_maps.append(
            {
                "xT": xT[c],
                "idx": idx_wrapped[c],
                "dloc": dlocT[c],
                "sel": selT[c],
                "iota": iota_np,
                "diag5": diag5_np,
                "id128": id_np,
                "dis": disT[c],
                "W1": W1_np,
                "W2": W2_np,
                "W3T": W3T_np,
                "b3": b3_np,
            }
        )
    return meta, in_maps


def build_nc(meta):
    NPC = meta["NPC"]
    NTILES = meta["NTILES"]
    NTAB = meta["NTAB"]
    NBUCKETS = meta["NBUCKETS"]
    BUCKET_ROWS = meta["BUCKET_ROWS"]
    NSG = meta["NSG"]
    KIN = meta["KIN"]
    F_IN = meta["F_IN"]
    HID = meta["HID"]
    NCLS = meta["NCLS"]
    TOT_IDX = meta["TOT_IDX"]
    TOT_CHUNKS = meta["TOT_CHUNKS"]
    MAXCH = meta["MAXCH"]
    calls = meta["calls"]
    tile_chunks = meta["tile_chunks"]

    nc = bacc.Bacc(
        "TRN2",
        target_bir_lowering=False,
        debug=False,
        num_devices=NCORES,
        dynamic_dma_scratch_size=65536,
        num_swdge_queues=4,
    )

    xT_d = nc.dram_tensor("xT", [F_IN, NPC], F16, kind="ExternalInput")
    idx_d = nc.dram_tensor("idx", [128, TOT_IDX // 16], mybir.dt.int16, kind="ExternalInput")
    dloc_d = nc.dram_tensor("dloc", [128, TOT_CHUNKS], F32, kind="ExternalInput")
    sel_d = nc.dram_tensor("sel", [128, TOT_CHUNKS * 128], F16, kind="ExternalInput")
    iota_d = nc.dram_tensor("iota", [128, 128], F16, kind="ExternalInput")
    diag5_d = nc.dram_tensor("diag5", [128, 128], F16, kind="ExternalInput")
    id_d = nc.dram_tensor("id128", [128, 128], F16, kind="ExternalInput")
    dis_d = nc.dram_tensor("dis", [128, NTILES], F32, kind="ExternalInput")
    W1_d = nc.dram_tensor("W1", [KIN, 128, HID], F16, kind="ExternalInput")
    W2_d = nc.dram_tensor("W2", [HID, HID], F16, kind="ExternalInput")
    W3T_d = nc.dram_tensor("W3T", [HID, NCLS], F16, kind="ExternalInput")
    b3_d = nc.dram_tensor("b3", [NCLS, 1], F32, kind="ExternalInput")
    out_d = nc.dram_tensor("out", [NCLS, NPC], F32, kind="ExternalOutput")

    zs1_own = nc.dram_tensor("zs1_own", [NPC, HID], F16)
    zs2_own = nc.dram_tensor("zs2_own", [NPC, HID], F16)
    tab1 = nc.dram_tensor("tab1", [NTAB, HID], F16, addr_space="Shared")
    tab2 = nc.dram_tensor("tab2", [NTAB, HID], F16, addr_space="Shared")

    sel_engines = {
        "v": nc.vector,
        "s": nc.any,
        "p": nc.gpsimd,
    }

    with tile.TileContext(nc) as tc:
        with (
            tc.tile_pool(name="const", bufs=1) as constp,
            tc.tile_pool(name="zs", bufs=1) as zsp,
            tc.tile_pool(name="meta", bufs=6) as metap,
            tc.tile_pool(name="epi", bufs=3) as epip,
            tc.tile_pool(name="agg", bufs=SG_TILES, space="PSUM") as aggp,
            tc.tile_pool(name="mpsum", bufs=2, space="PSUM") as mpsump,
        ):
            # xin is scoped to phase A (closed before the big gat/sel pools
            # open) so its SBUF is reused for a 4th msg buffer.
            _xin_cm = tc.tile_pool(name="xin", bufs=2)
            xinp = _xin_cm.__enter__()
            iota_t = constp.tile([128, 128], F16)
            nc.sync.dma_start(iota_t[:], iota_d[:])
            diag5_t = constp.tile([128, 128], F16)
            nc.sync.dma_start(diag5_t[:], diag5_d[:])
            id_t = constp.tile([128, 128], F16)
            nc.sync.dma_start(id_t[:], id_d[:])
            dis_t = constp.tile([128, NTILES], F32)
            nc.sync.dma_start(dis_t[:], dis_d[:])
            W1_t = constp.tile([128, KIN, HID], F16)
            nc.sync.dma_start(W1_t[:], W1_d.rearrange("k p h -> p k h"))
            W2_t = constp.tile([HID, HID], F16)
            nc.sync.dma_start(W2_t[:], W2_d[:])
            W3T_t = constp.tile([HID, NCLS], F16)
            nc.sync.dma_start(W3T_t[:], W3T_d[:])
            b3_t = constp.tile([NCLS, 1], F32)
            nc.sync.dma_start(b3_t[:], b3_d[:])

            zs1_all = zsp.tile([128, NTILES, HID], F16, tag="zs1")
            zs2_all = zsp.tile([128, NTILES, HID], F16, tag="zs2")

            xT_v = xT_d.rearrange("(k p) n -> k p n", p=128)
            zs1_v = zs1_own.rearrange("(g p) h -> g p h", p=128)
            zs2_v = zs2_own.rearrange("(g p) h -> g p h", p=128)

            # ---------------- phase A: zs1 = dis * (x @ W1) ----------------
            for s in range(NSG):
                t0 = s * SG_TILES
                nt = min(NTILES, t0 + SG_TILES) - t0
                xs = xinp.tile([128, KIN, SG_TILES * 128], F16, tag="xs")
                nc.sync.dma_start(
                    xs[:, :, : nt * 128],
                    xT_v[:, :, t0 * 128 : (t0 + nt) * 128].rearrange(
                        "k p n -> p k n"
                    ),
                )
                for i in range(nt):
                    t = t0 + i
                    z_ps = mpsump.tile([128, HID], F32, tag="mm")
                    for k in range(KIN):
                        nc.tensor.matmul(
                            z_ps[:],
                            xs[:, k, i * 128 : (i + 1) * 128],
                            W1_t[:, k, :],
                            start=(k == 0),
                            stop=(k == KIN - 1),
                        )
                    nc.scalar.activation(
                        zs1_all[:, t, :],
                        z_ps[:],
                        mybir.ActivationFunctionType.Copy,
                        bias=0.0,
                        scale=dis_t[:, t : t + 1],
                    )
                nc.sync.dma_start(
                    zs1_v[t0 : t0 + nt].rearrange("g p h -> p g h"),
                    zs1_all[:, t0 : t0 + nt, :],
                )

            nc.gpsimd.collective_compute(
                "AllGather",
                mybir.AluOpType.bypass,
                ins=[zs1_own[:]],
                outs=[tab1[:]],
                replica_groups=[list(range(NCORES))],
            )

            _xin_cm.__exit__(None, None, None)
            _gat_cm = tc.tile_pool(name="gat", bufs=8)
            gatp = _gat_cm.__enter__()
            _sel_cm = tc.tile_pool(name="sel", bufs=4)
            selp = _sel_cm.__enter__()

            # ---------------- agg layer (shared for both layers) -----------
            def agg_layer(tab_dram, zs_src_all, layer):
                """Aggregate per supergroup; returns per-tile epilogue hook."""
                parts = set(
                    os.environ.get("GCN_AGG_PARTS", "gather,sel,mm,epi").split(",")
                )
                sel_i = 0
                tile_seen = np.zeros(NTILES, np.int64)
                psums = {}
                qn = 0
                for s in range(NSG):
                    t0 = s * SG_TILES
                    nt = min(NTILES, t0 + SG_TILES) - t0
                    # self-loop first (opens accumulation)
                    for i in range(nt):
                        t = t0 + i
                        ps = aggp.tile([128, HID], F32, tag="agg")
                        psums[t] = ps
                        nc.tensor.matmul(
                            ps[:],
                            diag5_t[:],
                            zs_src_all[:, t, :],
                            start=True,
                            stop=("mm" not in parts),
                        )
                    sgcalls = [c for c in calls if c[0] == s]
                    for _, b, io, co, tl in sgcalls:
                        nch = sum(n for _, n in tl)
                        if nch == 0:
                            continue
                        L = nch * 128
                        idx_t = metap.tile(
                            [128, L // 16], mybir.dt.int16, tag="idx"
                        )
                        nc.scalar.dma_start(
                            idx_t[:], idx_d[:, io // 16 : (io + L) // 16]
                        )
                        # sel in 32-chunk subtiles: the first matmuls only
                        # wait on a 1 MB blob instead of the full 2 MB call.
                        sel_ts = []
                        for h0 in range(0, nch, 32):
                            hn = min(32, nch - h0)
                            st = selp.tile([128, 32 * 128], F16, tag="sel")
                            nc.sync.dma_start(
                                st[:, : hn * 128],
                                sel_d[:, (co + h0) * 128 : (co + h0 + hn) * 128],
                            )
                            sel_ts.append(st)
                        msg_t = gatp.tile([128, nch, HID], F16, tag="msg")
                        if "gather" in parts:
                            nc.gpsimd.dma_gather(
                                msg_t[:],
                                tab_dram[b * BUCKET_ROWS : (b + 1) * BUCKET_ROWS, :],
                                idx_t[:],
                                L,
                                L,
                                HID,
                                single_packet=False,
                                queue_num=qn,
                            )
                            qn = (qn + 1) % 4
                        else:
                            nc.gpsimd.memset(msg_t[:], 0.0)
                        j = 0
                        for t, n in tl:
                            for _ in range(n):
                                tile_seen[t] += 1
                                if "mm" in parts:
                                    nc.tensor.matmul(
                                        psums[t][:],
                                        sel_ts[j // 32][
                                            :, (j % 32) * 128 : (j % 32 + 1) * 128
                                        ],
                                        msg_t[:, j, :],
                                        start=False,
                                        stop=(tile_seen[t] == tile_chunks[t]),
                                    )
                                j += 1
                    # epilogue for this supergroup's tiles
                    for i in range(nt):
                        t = t0 + i
                        if "epi" in parts:
                            epilogue(t, psums.pop(t), layer)

            def epilogue(t, ps, layer):
                if layer == 1:
                    # h1 = relu(dis*ps); zs2 = dis * (h1 @ W2)
                    h_sb = epip.tile([128, HID], F16, tag="h")
                    nc.scalar.activation(
                        h_sb[:],
                        ps[:],
                        mybir.ActivationFunctionType.Relu,
                        bias=0.0,
                        scale=dis_t[:, t : t + 1],
                    )
                    tr_ps = mpsump.tile([128, 128], F16, tag="mm")
                    nc.tensor.transpose(tr_ps[:], h_sb[:], id_t[:])
                    hT_sb = epip.tile([128, 128], F16, tag="hT")
                    nc.vector.tensor_copy(hT_sb[:], tr_ps[:])
                    z_ps = mpsump.tile([128, HID], F32, tag="mm")
                    nc.tensor.matmul(z_ps[:], hT_sb[:], W2_t[:])
                    nc.scalar.activation(
                        zs2_all[:, t, :],
                        z_ps[:],
                        mybir.ActivationFunctionType.Copy,
                        bias=0.0,
                        scale=dis_t[:, t : t + 1],
                    )
                    s, i = t // SG_TILES, t % SG_TILES
                    if i == SG_TILES - 1 or t == NTILES - 1:
                        t0 = s * SG_TILES
                        nt = t - t0 + 1
                        nc.sync.dma_start(
                            zs2_v[t0 : t0 + nt].rearrange("g p h -> p g h"),
                            zs2_all[:, t0 : t0 + nt, :],
                        )
                else:
                    # h2 = dis*ps ; out = W3 @ h2.T + b3
                    h_sb = epip.tile([128, HID], F16, tag="h")
                    nc.scalar.activation(
                        h_sb[:],
                        ps[:],
                        mybir.ActivationFunctionType.Copy,
                        bias=0.0,
                        scale=dis_t[:, t : t + 1],
                    )
                    tr_ps = mpsump.tile([128, 128], F16, tag="mm")
                    nc.tensor.transpose(tr_ps[:], h_sb[:], id_t[:])
                    hT_sb = epip.tile([128, 128], F16, tag="hT")
                    nc.vector.tensor_copy(hT_sb[:], tr_ps[:])
                    o_ps = mpsump.tile([NCLS, 128], F32, tag="mm")
                    nc.tensor.matmul(o_ps[:], W3T_t[:], hT_sb[:])
                    o_sb = epip.tile([NCLS, 128], F32, tag="o")
                    nc.vector.tensor_scalar(
                        o_sb[:], o_ps[:], b3_t[:], None, mybir.AluOpType.add
                    )
                    nc.sync.dma_start(out_d[:, t * 128 : (t + 1) * 128], o_sb[:])

            dbg = int(os.environ.get("GCN_DEBUG_LEVEL", "3"))
            if dbg >= 2:
                agg_layer(tab1, zs1_all, layer=1)

            if dbg >= 3:
                nc.gpsimd.collective_compute(
                    "AllGather",
                    mybir.AluOpType.bypass,
                    ins=[zs2_own[:]],
                    outs=[tab2[:]],
                    replica_groups=[list(range(NCORES))],
                )

                agg_layer(tab2, zs2_all, layer=2)
            else:
                zt = epip.tile([NCLS, 128], F32, tag="o")
                nc.gpsimd.memset(zt[:], 0.0)
                for t in range(NTILES):
                    nc.sync.dma_start(out_d[:, t * 128 : (t + 1) * 128], zt[:])

            _sel_cm.__exit__(None, None, None)
            _gat_cm.__exit__(None, None, None)

    nc.compile()
    return nc


_PROFILE_HOOK_DONE = False


def _install_profile_hook():
    """The container's antenv lacks axon_hooks; inject it so trace=True works."""
    global _PROFILE_HOOK_DONE
    if _PROFILE_HOOK_DONE:
        return
    _PROFILE_HOOK_DONE = True
    import types

    try:
        from antenv.axon_hooks import get_axon_ntff_profile_hook  # noqa: F401

        return  # real module exists
    except ImportError:
        pass
    try:
        from trn_agent_boot.trn_boot import _ntff_profile_via_ctypes

        hook = _ntff_profile_via_ctypes("/opt/axon/libaxon_pjrt.so")
    except Exception:
        hook = None
    mod = types.ModuleType("antenv.axon_hooks")
    mod._hook = hook
    mod.set_axon_ntff_profile_hook = lambda h: setattr(mod, "_hook", h)
    mod.get_axon_ntff_profile_hook = lambda: mod._hook
    import antenv

    sys.modules["antenv.axon_hooks"] = mod
    antenv.axon_hooks = mod


def kernel(x, edge_index, W1, W2, W3, b3, trace=False):
    x = np.asarray(x)
    edge_index = np.asarray(edge_index)
    if trace:
        _install_profile_hook()
    meta, in_maps = preprocess(x, edge_index, W1, W2, W3, b3)
    nc = build_nc(meta)
    res = run_bass_kernel_spmd(nc, in_maps, list(range(NCORES)), trace=trace)
    outs = []
    for c in range(NCORES):
        o = res.results[c]["out"]  # [NCLS, NPC]
        outs.append(o.T[: meta["npc_raw"]])
    full = np.concatenate(outs, axis=0)[: meta["N"]]
    kernel.last_result = res
    return np.ascontiguousarray(full.astype(np.float32))


if __name__ == "__main__":
    # tiny self-test
    rng = np.random.default_rng(1)
    N, E, F, H, C = 2048, 16384, 512, 128, 16
    x = rng.standard_normal((N, F)).astype(np.float32)
    ei = rng.integers(0, N, (2, E)).astype(np.int32)
    W1 = (rng.standard_normal((F, H)) / np.sqrt(F)).astype(np.float32)
    W2 = (rng.standard_normal((H, H)) / np.sqrt(H)).astype(np.float32)
    W3 = (rng.standard_normal((C, H)) / np.sqrt(H)).astype(np.float32)
    b3 = np.zeros(C, np.float32)

    fill = float(np.trunc(np.log2(E / N)))
    deg = np.bincount(ei[0], minlength=N) + fill
    dis = 1.0 / np.sqrt(deg)

    def gcn(h, W):
        z = h @ W
        zs = dis[:, None] * z
        agg = np.zeros_like(zs)
        np.add.at(agg, ei[0], zs[ei[1]])
        return dis[:, None] * (agg + fill * zs)

    h = np.maximum(gcn(x, W1), 0.0)
    h = gcn(h, W2)
    expected = h @ W3.T + b3

    got = kernel(x, ei, W1, W2, W3, b3)
    err = np.abs(got - expected).max() / np.abs(expected).max()
    print(f"rel err: {err:.3e}")
    print("PASS" if err < 2e-2 else "FAIL")



# revision 36
# speedup vs baseline: 1.1731x; 1.1731x over previous
"""2-layer GCN (segment-sum message passing) on 8 trn2 NeuronCores.

Math (from the reference):
    row/col have self-loops appended with weight 5 (= trunc(log2(E/N))).
    deg[i] = (# real edges with row==i) + 5 ;  dis = 1/sqrt(deg)
    norm[e] = dis[row]*w*dis[col]  (w=1 for real edges)
    gcn(h, W): agg[d] = sum_{e: row=d} norm[e] * (h@W)[col[e]]
    out = (relu(gcn(x,W1)) -> gcn(.,W2)) @ W3.T + b3

Factorization used here: with zs = dis * (h@W),
    gcn(h,W)[d] = dis[d] * ( sum_{e: row=d} zs[col[e]]  +  5*zs[d] )
so the per-edge work is a pure gather of zs rows plus a segment-sum, with no
per-edge scaling. The segment-sum runs on the tensor engine as
Sel.T @ msg where Sel[e,:] is the one-hot of the edge's destination-within-
tile, built on-chip by tensor_scalar(is_equal) against an iota constant.
The 5*zs[d] self-loop term is a diag(5) matmul into the same PSUM
accumulator.

Sharding: destination nodes split across 8 cores; zs tables are
all-gathered between layers; gathers read the fp16 table via dma_gather
(int16 indices, 4 source-range buckets to stay under the 32768-row limit).
"""

import math
import os
import sys

sys.path.insert(0, "/opt/trn_rl_repo")

import numpy as np

import concourse.bass as bass  # noqa: F401
import concourse.tile as tile
from concourse import bacc, mybir
from concourse.bass_utils import run_bass_kernel_spmd

NCORES = 8
SG_TILES = 6  # dst tiles per supergroup (PSUM agg banks)
SEL_PATTERN = os.environ.get("GCN_SEL_PATTERN", "vvs")  # v=DVE s=ACT(any) p=POOL
F16 = mybir.dt.float16
F32 = mybir.dt.float32


def _ceil(a, b):
    return -(-a // b)


def preprocess(x, edge_index, W1, W2, W3, b3):
    """Host-side layout/index prep. Returns (meta, in_maps)."""
    N, F_IN = x.shape
    HID = W1.shape[1]
    NCLS = W3.shape[0]
    E = edge_index.shape[1]

    npc_raw = _ceil(N, NCORES)  # nodes per core (unpadded)
    NPC = _ceil(npc_raw, 128) * 128  # padded per-core rows
    NTILES = NPC // 128
    NTAB = NCORES * NPC  # table rows
    NBUCKETS = _ceil(NTAB, 25088)
    BUCKET_ROWS = _ceil(_ceil(NTAB, NBUCKETS), 128) * 128
    assert BUCKET_ROWS <= 32768
    NSG = _ceil(NTILES, SG_TILES)
    KIN = F_IN // 128  # contraction chunks for layer-1 matmul

    row = np.asarray(edge_index[0], np.int64)
    col = np.asarray(edge_index[1], np.int64)

    fill = float(math.trunc(math.log2(E / N)))
    deg = np.bincount(row, minlength=N).astype(np.float64) + fill
    dis = (1.0 / np.sqrt(deg)).astype(np.float32)
    dis_pad = np.zeros(NTAB, np.float32)
    tab_row_of_node = (np.arange(N) // npc_raw) * NPC + (np.arange(N) % npc_raw)
    dis_pad[tab_row_of_node] = dis

    # per-edge placement
    e_core = row // npc_raw
    e_local = row % npc_raw
    e_tile = e_local // 128
    e_dstloc = (e_local % 128).astype(np.float32)
    src_tab = (col // npc_raw) * NPC + (col % npc_raw)
    e_bucket = src_tab // BUCKET_ROWS
    e_idx16 = (src_tab % BUCKET_ROWS).astype(np.int16)

    # stream order per core: supergroup -> bucket -> tile-within-sg -> edges
    e_sg = e_tile // SG_TILES
    e_tsg = e_tile % SG_TILES
    group = ((e_sg * NBUCKETS + e_bucket) * SG_TILES + e_tsg).astype(np.int64)
    NGRP = NSG * NBUCKETS * SG_TILES

    # counts[c, g]
    counts = np.zeros((NCORES, NGRP), np.int64)
    flat = e_core * NGRP + group
    bc = np.bincount(flat, minlength=NCORES * NGRP)
    counts = bc.reshape(NCORES, NGRP)
    gcnt = counts.max(axis=0)  # uniform padded counts (in edges)
    gchunks = _ceil(gcnt, 128)  # chunks per group (uniform)
    gpad = gchunks * 128

    # group -> (sg, b, tsg); chunk offsets
    goff = np.zeros(NGRP + 1, np.int64)
    np.cumsum(gpad, out=goff[1:])
    TOT_IDX = int(goff[-1])
    TOT_CHUNKS = TOT_IDX // 128

    # assemble per-core streams
    order = np.argsort(flat, kind="stable")
    run_starts = np.concatenate(([0], np.cumsum(bc)[:-1]))
    within = np.empty(E, np.int64)
    within[order] = np.arange(E) - run_starts[flat[order]]
    # position of edge e in its group's output block
    pos = goff[group] + within
    idx_stream = np.zeros((NCORES, TOT_IDX), np.int16)
    dloc_stream = np.full((NCORES, TOT_IDX), -64.0, np.float32)
    idx_stream[e_core, pos] = e_idx16
    dloc_stream[e_core, pos] = e_dstloc

    # wrapped idx layout: within each gather call (= (sg,b) run of groups),
    # idx j -> [j%16, call_off/16 + j//16]. Calls are contiguous (sg,b)
    # blocks of SG_TILES groups, so wrapping per 16 within the whole stream
    # works as long as each call's length %16 == 0 (it is: %128).
    # call boundaries:
    # walrus caps one gather at 8192 indices (64 chunks): split bigger calls.
    MAXCH = 32
    calls = []  # (sg, b, idx_off, chunk_off, [(tile, nchunks), ...])
    for s in range(NSG):
        tiles = list(range(s * SG_TILES, min(NTILES, (s + 1) * SG_TILES)))
        for b in range(NBUCKETS):
            g0 = (s * NBUCKETS + b) * SG_TILES
            idx_off = int(goff[g0])
            assert idx_off % 128 == 0
            pieces = []  # flat (tile, nchunks) honoring MAXCH
            cur = []
            cur_n = 0
            for i, t in enumerate(tiles):
                n = int(gchunks[g0 + i])
                while n > 0:
                    take = min(n, MAXCH - cur_n)
                    if take > 0:
                        cur.append((t, take))
                        cur_n += take
                        n -= take
                    if cur_n == MAXCH:
                        pieces.append(cur)
                        cur = []
                        cur_n = 0
            if cur:
                pieces.append(cur)
            off = idx_off
            for tl in pieces:
                calls.append((s, b, off, off // 128, tl))
                off += sum(nn for _, nn in tl) * 128
            assert off == int(goff[g0 + len(tiles)])

    idx_wrapped = np.zeros((NCORES, 128, TOT_IDX // 16), np.int16)
    for s, b, io, co, tl in calls:
        L = sum(n for _, n in tl) * 128
        if L == 0:
            continue
        blk = idx_stream[:, io : io + L].reshape(NCORES, L // 16, 16)
        blk = np.swapaxes(blk, 1, 2)  # [NCORES, 16, L/16]
        idx_wrapped[:, :, io // 16 : (io + L) // 16] = np.tile(blk, (1, 8, 1))

    dlocT = np.ascontiguousarray(
        np.swapaxes(dloc_stream.reshape(NCORES, TOT_CHUNKS, 128), 1, 2)
    )  # [NCORES, 128, TOT_CHUNKS]

    # host-precomputed Sel one-hot stream: per call, layout [128 edge-lanes,
    # nch*128 dst-cols] contiguous so each call DMAs one contiguous blob.
    eye = np.eye(128, dtype=np.float16)
    zero_row = np.zeros(128, np.float16)
    sel_rows = np.concatenate([eye, zero_row[None, :]])  # [-1] -> zeros
    dl_int = np.where(
        dloc_stream < 0, 128, dloc_stream.astype(np.int64)
    )  # [NCORES, TOT_IDX]
    sel_full = sel_rows[dl_int]  # [NCORES, TOT_IDX, 128] fp16
    sel_full = sel_full.reshape(NCORES, TOT_CHUNKS, 128, 128)
    selT = np.ascontiguousarray(
        np.swapaxes(sel_full, 1, 2).reshape(NCORES, 128, TOT_CHUNKS * 128)
    )
    del sel_full

    # per-tile chunk totals (for start/stop flags)
    tile_chunks = np.zeros(NTILES, np.int64)
    for s, b, io, co, tl in calls:
        for t, n in tl:
            tile_chunks[t] += n

    # dense inputs
    xpad = np.zeros((NTAB, F_IN), np.float32)
    xpad[tab_row_of_node] = np.asarray(x, np.float32)
    xT = np.ascontiguousarray(
        np.swapaxes(xpad.reshape(NCORES, NPC, F_IN), 1, 2).astype(np.float16)
    )  # [NCORES, F_IN, NPC]

    disT = np.ascontiguousarray(
        np.swapaxes(dis_pad.reshape(NCORES, NTILES, 128), 1, 2)
    )  # [NCORES, 128, NTILES]

    iota_np = np.tile(np.arange(128, dtype=np.float16)[None, :], (128, 1))
    diag5_np = (fill * np.eye(128)).astype(np.float16)
    id_np = np.eye(128, dtype=np.float16)
    W1_np = np.ascontiguousarray(
        np.asarray(W1, np.float32).reshape(KIN, 128, HID).astype(np.float16)
    )
    W2_np = np.asarray(W2, np.float32).astype(np.float16)
    W3T_np = np.ascontiguousarray(np.asarray(W3, np.float32).T.astype(np.float16))
    b3_np = np.asarray(b3, np.float32).reshape(NCLS, 1)

    meta = dict(
        N=N,
        F_IN=F_IN,
        HID=HID,
        NCLS=NCLS,
        NPC=NPC,
        npc_raw=npc_raw,
        NTILES=NTILES,
        NTAB=NTAB,
        NBUCKETS=NBUCKETS,
        BUCKET_ROWS=BUCKET_ROWS,
        NSG=NSG,
        KIN=KIN,
        TOT_IDX=TOT_IDX,
        TOT_CHUNKS=TOT_CHUNKS,
        MAXCH=MAXCH,
        calls=calls,
        tile_chunks=tile_chunks,
    )
    in_maps = []
    for c in range(NCORES):
        in_maps.append(
            {
                "xT": xT[c],
                "idx": idx_wrapped[c],
                "dloc": dlocT[c],
                "sel": selT[c],
                "iota": iota_np,
                "diag5": diag5_np,
                "id128": id_np,
                "dis": disT[c],
                "W1": W1_np,
                "W2": W2_np,
                "W3T": W3T_np,
                "b3": b3_np,
            }
        )
    return meta, in_maps


def build_nc(meta):
    NPC = meta["NPC"]
    NTILES = meta["NTILES"]
    NTAB = meta["NTAB"]
    NBUCKETS = meta["NBUCKETS"]
    BUCKET_ROWS = meta["BUCKET_ROWS"]
    NSG = meta["NSG"]
    KIN = meta["KIN"]
    F_IN = meta["F_IN"]
    HID = meta["HID"]
    NCLS = meta["NCLS"]
    TOT_IDX = meta["TOT_IDX"]
    TOT_CHUNKS = meta["TOT_CHUNKS"]
    MAXCH = meta["MAXCH"]
    calls = meta["calls"]
    tile_chunks = meta["tile_chunks"]

    nc = bacc.Bacc(
        "TRN2",
        target_bir_lowering=False,
        debug=False,
        num_devices=NCORES,
        dynamic_dma_scratch_size=65536,
        num_swdge_queues=4,
    )

    xT_d = nc.dram_tensor("xT", [F_IN, NPC], F16, kind="ExternalInput")
    idx_d = nc.dram_tensor("idx", [128, TOT_IDX // 16], mybir.dt.int16, kind="ExternalInput")
    dloc_d = nc.dram_tensor("dloc", [128, TOT_CHUNKS], F32, kind="ExternalInput")
    sel_d = nc.dram_tensor("sel", [128, TOT_CHUNKS * 128], F16, kind="ExternalInput")
    iota_d = nc.dram_tensor("iota", [128, 128], F16, kind="ExternalInput")
    diag5_d = nc.dram_tensor("diag5", [128, 128], F16, kind="ExternalInput")
    id_d = nc.dram_tensor("id128", [128, 128], F16, kind="ExternalInput")
    dis_d = nc.dram_tensor("dis", [128, NTILES], F32, kind="ExternalInput")
    W1_d = nc.dram_tensor("W1", [KIN, 128, HID], F16, kind="ExternalInput")
    W2_d = nc.dram_tensor("W2", [HID, HID], F16, kind="ExternalInput")
    W3T_d = nc.dram_tensor("W3T", [HID, NCLS], F16, kind="ExternalInput")
    b3_d = nc.dram_tensor("b3", [NCLS, 1], F32, kind="ExternalInput")
    out_d = nc.dram_tensor("out", [NCLS, NPC], F32, kind="ExternalOutput")

    zs1_own = nc.dram_tensor("zs1_own", [NPC, HID], F16)
    zs2_own = nc.dram_tensor("zs2_own", [NPC, HID], F16)
    tab1 = nc.dram_tensor("tab1", [NTAB, HID], F16, addr_space="Shared")
    tab2 = nc.dram_tensor("tab2", [NTAB, HID], F16, addr_space="Shared")

    sel_engines = {
        "v": nc.vector,
        "s": nc.any,
        "p": nc.gpsimd,
    }

    with tile.TileContext(nc) as tc:
        with (
            tc.tile_pool(name="const", bufs=1) as constp,
            tc.tile_pool(name="zs", bufs=1) as zsp,
            tc.tile_pool(name="meta", bufs=6) as metap,
            tc.tile_pool(name="epi", bufs=3) as epip,
            tc.tile_pool(name="agg", bufs=SG_TILES, space="PSUM") as aggp,
            tc.tile_pool(name="mpsum", bufs=2, space="PSUM") as mpsump,
        ):
            # xin is scoped to phase A (closed before the big gat/sel pools
            # open) so its SBUF is reused for a 4th msg buffer.
            _xin_cm = tc.tile_pool(name="xin", bufs=2)
            xinp = _xin_cm.__enter__()
            iota_t = constp.tile([128, 128], F16)
            nc.sync.dma_start(iota_t[:], iota_d[:])
            diag5_t = constp.tile([128, 128], F16)
            nc.sync.dma_start(diag5_t[:], diag5_d[:])
            id_t = constp.tile([128, 128], F16)
            nc.sync.dma_start(id_t[:], id_d[:])
            dis_t = constp.tile([128, NTILES], F32)
            nc.sync.dma_start(dis_t[:], dis_d[:])
            W1_t = constp.tile([128, KIN, HID], F16)
            nc.sync.dma_start(W1_t[:], W1_d.rearrange("k p h -> p k h"))
            W2_t = constp.tile([HID, HID], F16)
            nc.sync.dma_start(W2_t[:], W2_d[:])
            W3T_t = constp.tile([HID, NCLS], F16)
            nc.sync.dma_start(W3T_t[:], W3T_d[:])
            b3_t = constp.tile([NCLS, 1], F32)
            nc.sync.dma_start(b3_t[:], b3_d[:])

            zs1_all = zsp.tile([128, NTILES, HID], F16, tag="zs1")
            zs2_all = zsp.tile([128, NTILES, HID], F16, tag="zs2")

            xT_v = xT_d.rearrange("(k p) n -> k p n", p=128)
            zs1_v = zs1_own.rearrange("(g p) h -> g p h", p=128)
            zs2_v = zs2_own.rearrange("(g p) h -> g p h", p=128)

            # ---------------- phase A: zs1 = dis * (x @ W1) ----------------
            for s in range(NSG):
                t0 = s * SG_TILES
                nt = min(NTILES, t0 + SG_TILES) - t0
                xs = xinp.tile([128, KIN, SG_TILES * 128], F16, tag="xs")
                nc.sync.dma_start(
                    xs[:, :, : nt * 128],
                    xT_v[:, :, t0 * 128 : (t0 + nt) * 128].rearrange(
                        "k p n -> p k n"
                    ),
                )
                for i in range(nt):
                    t = t0 + i
                    z_ps = mpsump.tile([128, HID], F32, tag="mm")
                    for k in range(KIN):
                        nc.tensor.matmul(
                            z_ps[:],
                            xs[:, k, i * 128 : (i + 1) * 128],
                            W1_t[:, k, :],
                            start=(k == 0),
                            stop=(k == KIN - 1),
                        )
                    nc.scalar.activation(
                        zs1_all[:, t, :],
                        z_ps[:],
                        mybir.ActivationFunctionType.Copy,
                        bias=0.0,
                        scale=dis_t[:, t : t + 1],
                    )
                nc.sync.dma_start(
                    zs1_v[t0 : t0 + nt].rearrange("g p h -> p g h"),
                    zs1_all[:, t0 : t0 + nt, :],
                )

            nc.gpsimd.collective_compute(
                "AllGather",
                mybir.AluOpType.bypass,
                ins=[zs1_own[:]],
                outs=[tab1[:]],
                replica_groups=[list(range(NCORES))],
            )

            _xin_cm.__exit__(None, None, None)
            _gat_cm = tc.tile_pool(name="gat", bufs=8)
            gatp = _gat_cm.__enter__()
            _sel_cm = tc.tile_pool(name="sel", bufs=4)
            selp = _sel_cm.__enter__()

            # ---------------- agg layer (shared for both layers) -----------
            def agg_layer(tab_dram, zs_src_all, layer):
                """Aggregate per supergroup; returns per-tile epilogue hook."""
                parts = set(
                    os.environ.get("GCN_AGG_PARTS", "gather,sel,mm,epi").split(",")
                )
                sel_i = 0
                tile_seen = np.zeros(NTILES, np.int64)
                psums = {}
                qn = 0
                for s in range(NSG):
                    t0 = s * SG_TILES
                    nt = min(NTILES, t0 + SG_TILES) - t0
                    # self-loop first (opens accumulation)
                    for i in range(nt):
                        t = t0 + i
                        ps = aggp.tile([128, HID], F32, tag="agg")
                        psums[t] = ps
                        nc.tensor.matmul(
                            ps[:],
                            diag5_t[:],
                            zs_src_all[:, t, :],
                            start=True,
                            stop=("mm" not in parts),
                        )
                    sgcalls = [c for c in calls if c[0] == s]
                    for _, b, io, co, tl in sgcalls:
                        nch = sum(n for _, n in tl)
                        if nch == 0:
                            continue
                        L = nch * 128
                        idx_t = metap.tile(
                            [128, L // 16], mybir.dt.int16, tag="idx"
                        )
                        nc.scalar.dma_start(
                            idx_t[:], idx_d[:, io // 16 : (io + L) // 16]
                        )
                        # sel in 32-chunk subtiles: the first matmuls only
                        # wait on a 1 MB blob instead of the full 2 MB call.
                        sel_ts = []
                        for h0 in range(0, nch, 32):
                            hn = min(32, nch - h0)
                            st = selp.tile([128, 32 * 128], F16, tag="sel")
                            nc.sync.dma_start(
                                st[:, : hn * 128],
                                sel_d[:, (co + h0) * 128 : (co + h0 + hn) * 128],
                            )
                            sel_ts.append(st)
                        msg_t = gatp.tile([128, nch, HID], F16, tag="msg")
                        if "gather" in parts:
                            nc.gpsimd.dma_gather(
                                msg_t[:],
                                tab_dram[b * BUCKET_ROWS : (b + 1) * BUCKET_ROWS, :],
                                idx_t[:],
                                L,
                                L,
                                HID,
                                single_packet=False,
                                queue_num=qn,
                            )
                            qn = (qn + 1) % 4
                        else:
                            nc.gpsimd.memset(msg_t[:], 0.0)
                        j = 0
                        for t, n in tl:
                            for _ in range(n):
                                tile_seen[t] += 1
                                if "mm" in parts:
                                    nc.tensor.matmul(
                                        psums[t][:],
                                        sel_ts[j // 32][
                                            :, (j % 32) * 128 : (j % 32 + 1) * 128
                                        ],
                                        msg_t[:, j, :],
                                        start=False,
                                        stop=(tile_seen[t] == tile_chunks[t]),
                                    )
                                j += 1
                    # epilogue for this supergroup's tiles
                    for i in range(nt):
                        t = t0 + i
                        if "epi" in parts:
                            epilogue(t, psums.pop(t), layer)

            def epilogue(t, ps, layer):
                if layer == 1:
                    # h1 = relu(dis*ps); zs2 = dis * (h1 @ W2)
                    h_sb = epip.tile([128, HID], F16, tag="h")
                    nc.scalar.activation(
                        h_sb[:],
                        ps[:],
                        mybir.ActivationFunctionType.Relu,
                        bias=0.0,
                        scale=dis_t[:, t : t + 1],
                    )
                    tr_ps = mpsump.tile([128, 128], F16, tag="mm")
                    nc.tensor.transpose(tr_ps[:], h_sb[:], id_t[:])
                    hT_sb = epip.tile([128, 128], F16, tag="hT")
                    nc.vector.tensor_copy(hT_sb[:], tr_ps[:])
                    z_ps = mpsump.tile([128, HID], F32, tag="mm")
                    nc.tensor.matmul(z_ps[:], hT_sb[:], W2_t[:])
                    nc.scalar.activation(
                        zs2_all[:, t, :],
                        z_ps[:],
                        mybir.ActivationFunctionType.Copy,
                        bias=0.0,
                        scale=dis_t[:, t : t + 1],
                    )
                    s, i = t // SG_TILES, t % SG_TILES
                    if i == SG_TILES - 1 or t == NTILES - 1:
                        t0 = s * SG_TILES
                        nt = t - t0 + 1
                        nc.sync.dma_start(
                            zs2_v[t0 : t0 + nt].rearrange("g p h -> p g h"),
                            zs2_all[:, t0 : t0 + nt, :],
                        )
                else:
                    # h2 = dis*ps ; out = W3 @ h2.T + b3
                    h_sb = epip.tile([128, HID], F16, tag="h")
                    nc.scalar.activation(
                        h_sb[:],
                        ps[:],
                        mybir.ActivationFunctionType.Copy,
                        bias=0.0,
                        scale=dis_t[:, t : t + 1],
                    )
                    tr_ps = mpsump.tile([128, 128], F16, tag="mm")
                    nc.tensor.transpose(tr_ps[:], h_sb[:], id_t[:])
                    hT_sb = epip.tile([128, 128], F16, tag="hT")
                    nc.vector.tensor_copy(hT_sb[:], tr_ps[:])
                    o_ps = mpsump.tile([NCLS, 128], F32, tag="mm")
                    nc.tensor.matmul(o_ps[:], W3T_t[:], hT_sb[:])
                    o_sb = epip.tile([NCLS, 128], F32, tag="o")
                    nc.vector.tensor_scalar(
                        o_sb[:], o_ps[:], b3_t[:], None, mybir.AluOpType.add
                    )
                    nc.sync.dma_start(out_d[:, t * 128 : (t + 1) * 128], o_sb[:])

            dbg = int(os.environ.get("GCN_DEBUG_LEVEL", "3"))
            if dbg >= 2:
                agg_layer(tab1, zs1_all, layer=1)

            if dbg >= 3:
                nc.gpsimd.collective_compute(
                    "AllGather",
                    mybir.AluOpType.bypass,
                    ins=[zs2_own[:]],
                    outs=[tab2[:]],
                    replica_groups=[list(range(NCORES))],
                )

                agg_layer(tab2, zs2_all, layer=2)
            else:
                zt = epip.tile([NCLS, 128], F32, tag="o")
                nc.gpsimd.memset(zt[:], 0.0)
                for t in range(NTILES):
                    nc.sync.dma_start(out_d[:, t * 128 : (t + 1) * 128], zt[:])

            _sel_cm.__exit__(None, None, None)
            _gat_cm.__exit__(None, None, None)

    nc.compile()
    return nc


_PROFILE_HOOK_DONE = False


def _install_profile_hook():
    """The container's antenv lacks axon_hooks; inject it so trace=True works."""
    global _PROFILE_HOOK_DONE
    if _PROFILE_HOOK_DONE:
        return
    _PROFILE_HOOK_DONE = True
    import types

    try:
        from antenv.axon_hooks import get_axon_ntff_profile_hook  # noqa: F401

        return  # real module exists
    except ImportError:
        pass
    try:
        from trn_agent_boot.trn_boot import _ntff_profile_via_ctypes

        hook = _ntff_profile_via_ctypes("/opt/axon/libaxon_pjrt.so")
    except Exception:
        hook = None
    mod = types.ModuleType("antenv.axon_hooks")
    mod._hook = hook
    mod.set_axon_ntff_profile_hook = lambda h: setattr(mod, "_hook", h)
    mod.get_axon_ntff_profile_hook = lambda: mod._hook
    import antenv

    sys.modules["antenv.axon_hooks"] = mod
    antenv.axon_hooks = mod


def kernel(x, edge_index, W1, W2, W3, b3, trace=False):
    x = np.asarray(x)
    edge_index = np.asarray(edge_index)
    if trace:
        _install_profile_hook()
    meta, in_maps = preprocess(x, edge_index, W1, W2, W3, b3)
    nc = build_nc(meta)
    res = run_bass_kernel_spmd(nc, in_maps, list(range(NCORES)), trace=trace)
    outs = []
    for c in range(NCORES):
        o = res.results[c]["out"]  # [NCLS, NPC]
        outs.append(o.T[: meta["npc_raw"]])
    full = np.concatenate(outs, axis=0)[: meta["N"]]
    kernel.last_result = res
    return np.ascontiguousarray(full.astype(np.float32))


if __name__ == "__main__":
    # tiny self-test
    rng = np.random.default_rng(1)
    N, E, F, H, C = 2048, 16384, 512, 128, 16
    x = rng.standard_normal((N, F)).astype(np.float32)
    ei = rng.integers(0, N, (2, E)).astype(np.int32)
    W1 = (rng.standard_normal((F, H)) / np.sqrt(F)).astype(np.float32)
    W2 = (rng.standard_normal((H, H)) / np.sqrt(H)).astype(np.float32)
    W3 = (rng.standard_normal((C, H)) / np.sqrt(H)).astype(np.float32)
    b3 = np.zeros(C, np.float32)

    fill = float(np.trunc(np.log2(E / N)))
    deg = np.bincount(ei[0], minlength=N) + fill
    dis = 1.0 / np.sqrt(deg)

    def gcn(h, W):
        z = h @ W
        zs = dis[:, None] * z
        agg = np.zeros_like(zs)
        np.add.at(agg, ei[0], zs[ei[1]])
        return dis[:, None] * (agg + fill * zs)

    h = np.maximum(gcn(x, W1), 0.0)
    h = gcn(h, W2)
    expected = h @ W3.T + b3

    got = kernel(x, ei, W1, W2, W3, b3)
    err = np.abs(got - expected).max() / np.abs(expected).max()
    print(f"rel err: {err:.3e}")
    print("PASS" if err < 2e-2 else "FAIL")

